# revision 58
# baseline (speedup 1.0000x reference)
"""Trainium2 Bass kernel for nn_ComposedCliffordSteerableKernel.

Computation (see reference): for each of 16x16 (m, n) block pairs, a tiny
3D conv (8,8,7^3) x (8,8,7^3) -> (8,8,7^3) with SAME padding, then
elementwise * shell * factor.

Scheme ("band16"): the TRN2 cost of a matmul is free_size x pe_cycle x
cycles_per_row, independent of how many partitions/columns are used.  So
the kernel packs everything except the (oh, ow, p) free rows into the
partition/column dims:

  psum[(n2,q,od), (oh,ow,p)] += sum_{(n2,j,dd)}
      W[(n2,j,dd), (kh,kw), (n2,q,od)] * k1t[(n2,j,dd), (oh+kh-3, ow+kw-3, p)]

- columns = (pair n2, out-blade q, out-depth od) = 112 of 128
- contraction = (pair n2, in-blade j, ABSOLUTE in-depth dd) = 112
- W is a banded-Toeplitz expansion of k2: W[.., dd, .., od] =
  k2[q, j, dd-od+3, kh, kw] (zero outside the band / across pairs),
  prepared on the host.  The whole depth-tap (kd) sum collapses into the
  dd contraction, so only 49 matmuls per (m, pair-group) remain: one per
  (kh, kw), each streaming (oh,ow,p) rows windowed to the valid
  oh in [max(0,3-kh), min(7,10-kh)) x ow window (rows outside the window
  get zero contribution from this tap).  Tap (3,3) has a full window and
  goes first (start=True initializes the whole accumulator).

Per core: 2 m-blocks x 8 pair-groups = 16 iterations, 49 matmuls each =
784 matmuls, ~175K charged PE rows (~73 us at 2.4 GHz).  All operands
fp16 (1 cycle/row), host-pre-transposed so every DMA is a contiguous
[112, X] block load (the BIR verifier requires the stationary matmul AP
to have a single free dim, so the 112 columns of each tap are stored
contiguously, cross-pair zeros included).  A chained burst of dummy
matmuls while the first weight DMA is in flight ramps the PE to full
clock.  Engine assignment keeps every queue unblocked: weights stream
on SP (pure prefetch, depth 6), k1/shell on Activation, out-DMAs on
Pool/SWDGE (an SP out-DMA would block in-order on the evacuation sem
and collapse the weight prefetch) with the final one on the by-then
idle SP.  shell*factor is folded on the host; one DVE multiply per
iteration evacuates PSUM through 6 rotating banks.  Sharding: core c
takes m-blocks 2c, 2c+1 (output rows 16c..16c+16); no inter-core
communication.  TimelineSim: 84.5 us vs 5469 us baseline (64.7x).
"""

import sys

for _p in ("/opt/trn_rl_repo",):
    if _p not in sys.path:
        sys.path.insert(0, _p)

import numpy as np

NB = 8
KS = 7
S3 = KS * KS * KS          # 343
N_CORES = 8
M_PER_CORE = 2
NIT = 16                   # (m2, pg) iterations per core
NPART = 112                # (n2, j|q, dd|od)
NFREE = KS * KS * NB       # 392 = (oh, ow, p)
NTAP = KS * KS             # 49 (kh, kw)

MODE = "band16"

_CACHE = {}


def _taps():
    """(kh, kw) order: full-window (3,3) first so start=True covers the
    whole accumulator; the rest in raster order."""
    rest = [(kh, kw) for kh in range(KS) for kw in range(KS)
            if (kh, kw) != (3, 3)]
    return [(3, 3)] + rest


def _build_nc(mode):
    import concourse.tile as tile
    from concourse import bacc, mybir

    f16 = mybir.dt.float16
    f32 = mybir.dt.float32

    nc = bacc.Bacc("TRN2", target_bir_lowering=False, debug=False)

    wdram = nc.dram_tensor("w", [NIT, NPART, NTAP * NPART], f16,
                           kind="ExternalInput")
    k1dram = nc.dram_tensor("k1t", [NIT, NPART, NFREE], f16,
                            kind="ExternalInput")
    shdram = nc.dram_tensor("sh", [NIT, NPART, NFREE], f16,
                            kind="ExternalInput")
    odram = nc.dram_tensor("out", [NIT, NPART, NFREE], f16,
                           kind="ExternalOutput")

    taps = _taps()
    WBUFS = 6
    NWARM = 12

    with tile.TileContext(nc) as tc:
        with (
            tc.tile_pool(name="w", bufs=WBUFS) as wpool,
            tc.tile_pool(name="io", bufs=4) as iopool,
            tc.tile_pool(name="ps", bufs=6, space="PSUM") as pspool,
            tc.tile_pool(name="wm", bufs=1) as warmpool,
            tc.tile_pool(name="wps", bufs=1, space="PSUM") as wpspool,
        ):
            # PE clock warmup: dummy matmuls on a zero tile while the
            # first weight DMA is in flight (result never read); memset
            # via Pool to keep DVE free
            warm = warmpool.tile([128, 512], f16, tag="warm")
            nc.gpsimd.memset(warm[:, :], 0.0)
            wps = wpspool.tile([128, 512], f32, tag="wps")
            for r in range(NWARM):
                # one long accumulation chain: no intermediate sems, so
                # the PE stays back-to-back busy and actually ramps
                nc.tensor.matmul(wps[:, :], warm[:, 0:128], warm[:, :],
                                 start=(r == 0), stop=(r == NWARM - 1))

            for it in range(NIT):
                # weight tile free layout: (tap, (n2',q,od)) — the 112
                # columns of each tap are contiguous, so the matmul's
                # stationary AP has a single free dim (BIR requirement)
                wt = wpool.tile([NPART, NTAP, NPART], f16, tag="w")
                nc.sync.dma_start(
                    out=wt.rearrange("c a b -> c (a b)"),
                    in_=wdram[it, :, :],
                )
                k1t = iopool.tile([NPART, KS, KS, NB], f16, tag="k1")
                nc.scalar.dma_start(
                    out=k1t.rearrange("c a b p -> c (a b p)"),
                    in_=k1dram[it, :, :],
                )
                sht = iopool.tile([NPART, NFREE], f16, tag="sh")
                nc.scalar.dma_start(out=sht[:, :], in_=shdram[it, :, :])

                ps = pspool.tile([NPART, NFREE], f32, tag="ps")
                psv = ps.rearrange("c (oh ow p) -> c oh ow p", oh=KS, ow=KS)
                for i, (kh, kw) in enumerate(taps):
                    oh0, oh1 = max(0, 3 - kh), min(KS, 10 - kh)
                    ow0, ow1 = max(0, 3 - kw), min(KS, 10 - kw)
                    nc.tensor.matmul(
                        psv[:, oh0:oh1, ow0:ow1, :],
                        wt[:, i, :],         # tap axis is execution-ordered
                        k1t[:, oh0 + kh - 3:oh1 + kh - 3,
                            ow0 + kw - 3:ow1 + kw - 3, :],
                        start=(i == 0),
                        stop=(i == len(taps) - 1),
                    )

                ot = iopool.tile([NPART, NFREE], f16, tag="ot")
                nc.vector.tensor_mul(ot[:, :], ps[:, :], sht[:, :])
                # out DMAs go via Pool/SWDGE: SP.SEQ stays a pure weight
                # prefetch stream (an SP out-DMA would block in-order on
                # the evacuation sem and collapse the prefetch depth).
                # The final one takes the faster HWDGE path on the by-then
                # idle SP to shorten the drain tail.
                if it == NIT - 1:
                    nc.sync.dma_start(out=odram[it, :, :], in_=ot[:, :])
                else:
                    nc.gpsimd.dma_start(out=odram[it, :, :], in_=ot[:, :])
    nc.compile()
    return nc


def _get_nc(mode=None):
    if mode is None:
        mode = MODE
    if mode not in _CACHE:
        _CACHE[mode] = _build_nc(mode)
    return _CACHE[mode]


def _host_prep(k1, k2, shell, factor):
    """Build the per-core DMA images (see module docstring for layouts)."""
    k1 = np.asarray(k1, np.float32).reshape(128, 128, KS, KS, KS)
    k2 = np.asarray(k2, np.float32).reshape(128, 128, KS, KS, KS)
    shell = np.asarray(shell, np.float32).reshape(128, 128, KS, KS, KS)
    fac = np.float32(np.asarray(factor).reshape(-1)[0])

    # k1 image: [m, pg, (n2,j,dd), (h,w,p)]
    A = k1.reshape(16, NB, NB, 2, NB, KS, KS, KS)      # m,p,pg,n2,j,d,h,w
    K1 = np.ascontiguousarray(
        A.transpose(0, 2, 3, 4, 5, 6, 7, 1)            # m,pg,n2,j,d,h,w,p
    ).reshape(16, NB, NPART, NFREE).astype(np.float16)

    # banded-Toeplitz k2 image: [m, pg, (n2,j,dd), (kh,kw), (n2',q,od)]
    B = k2.reshape(16, NB, NB, 2, NB, KS, KS, KS)      # m,q,pg,n2,j,td,kh,kw
    dd = np.arange(KS)[:, None]
    od = np.arange(KS)[None, :]
    td = dd - od + 3                                   # (dd, od)
    mask = ((td >= 0) & (td < KS)).astype(np.float32)
    tdc = np.clip(td, 0, KS - 1)
    WB = B[:, :, :, :, :, tdc, :, :]                   # m,q,pg,n2,j,dd,od,kh,kw
    WB = WB * mask[None, None, None, None, None, :, :, None, None]
    WBt = WB.transpose(0, 2, 3, 4, 5, 7, 8, 1, 6)      # m,pg,n2,j,dd,kh,kw,q,od
    # full weight image: part=(n2,j,dd), free=(tap, (n2',q,od)) with the
    # tap axis in EXECUTION order (see _taps) and zero cross-pair blocks
    WBt = WBt.reshape(16, NB, 2, NB, KS, NTAP, NB, KS)  # ..,dd,(khkw),q,od
    order = [kh * KS + kw for kh, kw in _taps()]
    WBt = WBt[:, :, :, :, :, order].astype(np.float16)
    Wfull = np.zeros((16, NB, 2, NB, KS, NTAP, 2, NB, KS), np.float16)
    for i in range(2):
        Wfull[:, :, i, :, :, :, i] = WBt[:, :, i]
    W = Wfull.reshape(16, NB, NPART, NTAP * NPART)

    # shell*factor image: [m, pg, (n2,q,od), (oh,ow,p)] fp16
    C = shell.reshape(16, NB, NB, 2, NB, KS, KS, KS)   # m,p,pg,n2,q,od,oh,ow
    SH = np.ascontiguousarray(
        C.transpose(0, 2, 3, 4, 5, 6, 7, 1) * fac      # m,pg,n2,q,od,oh,ow,p
    ).reshape(16, NB, NPART, NFREE).astype(np.float16)

    return W, K1, SH


def _make_in_maps(W, K1, SH):
    maps = []
    for c in range(N_CORES):
        sl = slice(2 * c, 2 * c + 2)
        maps.append({
            "w": np.ascontiguousarray(W[sl]).reshape(NIT, NPART, -1),
            "k1t": np.ascontiguousarray(K1[sl]).reshape(NIT, NPART, NFREE),
            "sh": np.ascontiguousarray(SH[sl]).reshape(NIT, NPART, NFREE),
        })
    return maps


def _gather(results):
    outs = [np.asarray(r["out"]) for r in results]      # each [16, 112, 392]
    full = np.stack(outs, axis=0).reshape(
        N_CORES, 2, NB, 2, NB, KS, KS, KS, NB
    )                                                   # c,m2,pg,n2,q,od,oh,ow,p
    full = full.transpose(0, 1, 8, 2, 3, 4, 5, 6, 7)    # c,m2,p,pg,n2,q,od,oh,ow
    return np.ascontiguousarray(full).reshape(
        128, 128, KS, KS, KS
    ).astype(np.float32)


def kernel(k1, k2, shell, factor, _trace=False):
    from concourse.bass_utils import run_bass_kernel_spmd

    nc = _get_nc(MODE)
    W, K1, SH = _host_prep(k1, k2, shell, factor)
    in_maps = _make_in_maps(W, K1, SH)
    try:
        res = run_bass_kernel_spmd(
            nc, in_maps, core_ids=list(range(N_CORES)), trace=_trace
        )
    except ModuleNotFoundError:
        res = run_bass_kernel_spmd(
            nc, in_maps, core_ids=list(range(N_CORES)), trace=False
        )
    out = _gather(res.results)
    if _trace:
        return out, res
    return out


# revision 62
# speedup vs baseline: 1.0027x; 1.0027x over previous
"""Trainium2 Bass kernel for nn_ComposedCliffordSteerableKernel.

Computation (see reference): for each of 16x16 (m, n) block pairs, a tiny
3D conv (8,8,7^3) x (8,8,7^3) -> (8,8,7^3) with SAME padding, then
elementwise * shell * factor.

Scheme ("band16"): the TRN2 cost of a matmul is free_size x pe_cycle x
cycles_per_row, independent of how many partitions/columns are used.  So
the kernel packs everything except the (oh, ow, p) free rows into the
partition/column dims:

  psum[(n2,q,od), (oh,ow,p)] += sum_{(n2,j,dd)}
      W[(n2,j,dd), (kh,kw), (n2,q,od)] * k1t[(n2,j,dd), (oh+kh-3, ow+kw-3, p)]

- columns = (pair n2, out-blade q, out-depth od) = 112 of 128
- contraction = (pair n2, in-blade j, ABSOLUTE in-depth dd) = 112
- W is a banded-Toeplitz expansion of k2: W[.., dd, .., od] =
  k2[q, j, dd-od+3, kh, kw] (zero outside the band / across pairs),
  prepared on the host.  The whole depth-tap (kd) sum collapses into the
  dd contraction, so only 49 matmuls per (m, pair-group) remain: one per
  (kh, kw), each streaming (oh,ow,p) rows windowed to the valid
  oh in [max(0,3-kh), min(7,10-kh)) x ow window (rows outside the window
  get zero contribution from this tap).  Tap (3,3) has a full window and
  goes first (start=True initializes the whole accumulator).

Per core: 2 m-blocks x 8 pair-groups = 16 iterations, 49 matmuls each =
784 matmuls, ~175K charged PE rows (~73 us at 2.4 GHz).  All operands
fp16 (1 cycle/row), host-pre-transposed so every DMA is a contiguous
[112, X] block load (the BIR verifier requires the stationary matmul AP
to have a single free dim, so the 112 columns of each tap are stored
contiguously, cross-pair zeros included).  A chained burst of dummy
matmuls while the first weight DMA is in flight ramps the PE to full
clock.  Engine assignment keeps every queue unblocked: weights stream
on SP (pure prefetch, depth 6), k1/shell on Activation, out-DMAs on
Pool/SWDGE (an SP out-DMA would block in-order on the evacuation sem
and collapse the weight prefetch) with the final one on the by-then
idle SP.  shell*factor is folded on the host; one DVE multiply per
iteration evacuates PSUM through 6 rotating banks.  Sharding: core c
takes m-blocks 2c, 2c+1 (output rows 16c..16c+16); no inter-core
communication.  TimelineSim: 84.5 us vs 5469 us baseline (64.7x).
"""

import sys

for _p in ("/opt/trn_rl_repo",):
    if _p not in sys.path:
        sys.path.insert(0, _p)

import numpy as np

NB = 8
KS = 7
S3 = KS * KS * KS          # 343
N_CORES = 8
M_PER_CORE = 2
NIT = 16                   # (m2, pg) iterations per core
NPART = 112                # (n2, j|q, dd|od)
NFREE = KS * KS * NB       # 392 = (oh, ow, p)
NTAP = KS * KS             # 49 (kh, kw)

MODE = "band16"

_CACHE = {}


def _taps():
    """(kh, kw) order: full-window (3,3) first so start=True covers the
    whole accumulator; the rest in raster order."""
    rest = [(kh, kw) for kh in range(KS) for kw in range(KS)
            if (kh, kw) != (3, 3)]
    return [(3, 3)] + rest


def _build_nc(mode):
    import concourse.tile as tile
    from concourse import bacc, mybir

    f16 = mybir.dt.float16
    f32 = mybir.dt.float32

    nc = bacc.Bacc("TRN2", target_bir_lowering=False, debug=False)

    wdram = nc.dram_tensor("w", [NIT, NPART, NTAP * NPART], f16,
                           kind="ExternalInput")
    k1dram = nc.dram_tensor("k1t", [NIT, NPART, NFREE], f16,
                            kind="ExternalInput")
    shdram = nc.dram_tensor("sh", [NIT, NPART, NFREE], f16,
                            kind="ExternalInput")
    odram = nc.dram_tensor("out", [NIT, NPART, NFREE], f16,
                           kind="ExternalOutput")

    taps = _taps()
    WBUFS = 6
    NWARM = 12

    with tile.TileContext(nc) as tc:
        with (
            tc.tile_pool(name="w", bufs=WBUFS) as wpool,
            tc.tile_pool(name="io", bufs=4) as iopool,
            tc.tile_pool(name="ps", bufs=5, space="PSUM") as pspool,
            tc.tile_pool(name="wm", bufs=1) as warmpool,
            tc.tile_pool(name="wps", bufs=1, space="PSUM") as wpspool,
            tc.tile_pool(name="psab", bufs=1, space="PSUM") as psabpool,
        ):
            # PE clock warmup: dummy matmuls on a zero tile while the
            # first weight DMA is in flight (result never read); memset
            # via Pool to keep DVE free
            warm = warmpool.tile([128, 512], f16, tag="warm")
            nc.gpsimd.memset(warm[:, :], 0.0)
            wps = wpspool.tile([128, 512], f32, tag="wps")
            for r in range(NWARM):
                # one long accumulation chain: no intermediate sems, so
                # the PE stays back-to-back busy and actually ramps
                nc.tensor.matmul(wps[:, :], warm[:, 0:128], warm[:, :],
                                 start=(r == 0), stop=(r == NWARM - 1))

            for it in range(NIT):
                # weight tile free layout: (tap, (n2',q,od)) — the 112
                # columns of each tap are contiguous, so the matmul's
                # stationary AP has a single free dim (BIR requirement)
                wt = wpool.tile([NPART, NTAP, NPART], f16, tag="w")
                nc.sync.dma_start(
                    out=wt.rearrange("c a b -> c (a b)"),
                    in_=wdram[it, :, :],
                )
                k1t = iopool.tile([NPART, KS, KS, NB], f16, tag="k1")
                nc.scalar.dma_start(
                    out=k1t.rearrange("c a b p -> c (a b p)"),
                    in_=k1dram[it, :, :],
                )
                sht = iopool.tile([NPART, NFREE], f16, tag="sh")
                nc.scalar.dma_start(out=sht[:, :], in_=shdram[it, :, :])

                if it < NIT - 1:
                    ps = pspool.tile([NPART, NFREE], f32, tag="ps")
                    psv = ps.rearrange("c (oh ow p) -> c oh ow p",
                                       oh=KS, ow=KS)
                    for i, (kh, kw) in enumerate(taps):
                        oh0, oh1 = max(0, 3 - kh), min(KS, 10 - kh)
                        ow0, ow1 = max(0, 3 - kw), min(KS, 10 - kw)
                        nc.tensor.matmul(
                            psv[:, oh0:oh1, ow0:ow1, :],
                            wt[:, i, :],     # tap axis is execution-ordered
                            k1t[:, oh0 + kh - 3:oh1 + kh - 3,
                                ow0 + kw - 3:ow1 + kw - 3, :],
                            start=(i == 0),
                            stop=(i == len(taps) - 1),
                        )
                    ot = iopool.tile([NPART, NFREE], f16, tag="ot")
                    nc.vector.tensor_mul(ot[:, :], ps[:, :], sht[:, :])
                    # out DMAs go via Pool/SWDGE: SP.SEQ stays a pure
                    # weight prefetch stream (an SP out-DMA would block
                    # in-order on the evacuation sem and collapse the
                    # prefetch depth)
                    nc.gpsimd.dma_start(out=odram[it, :, :], in_=ot[:, :])
                else:
                    # last iteration: split into oh-half accumulation
                    # chains A (oh 0:4) and B (oh 4:7), all A matmuls
                    # first — A's evacuation+store overlap B's compute so
                    # the drain tail only pays for the small B half
                    OHA = 4
                    FA, FB = OHA * KS * NB, (KS - OHA) * KS * NB
                    psa = psabpool.tile([NPART, FA], f32, tag="psa")
                    psb = psabpool.tile([NPART, FB], f32, tag="psb")
                    pva = psa.rearrange("c (oh ow p) -> c oh ow p", oh=OHA,
                                        ow=KS)
                    pvb = psb.rearrange("c (oh ow p) -> c oh ow p",
                                        oh=KS - OHA, ow=KS)
                    halves = []
                    for lo, hi, pv, base in ((0, OHA, pva, 0),
                                             (OHA, KS, pvb, OHA)):
                        sub = []
                        for i, (kh, kw) in enumerate(taps):
                            oh0 = max(max(0, 3 - kh), lo)
                            oh1 = min(min(KS, 10 - kh), hi)
                            if oh0 >= oh1:
                                continue
                            ow0, ow1 = max(0, 3 - kw), min(KS, 10 - kw)
                            sub.append((i, kh, kw, oh0, oh1, ow0, ow1))
                        halves.append((pv, base, sub))
                    for pv, base, sub in halves:
                        for k, (i, kh, kw, oh0, oh1, ow0, ow1) in                                 enumerate(sub):
                            nc.tensor.matmul(
                                pv[:, oh0 - base:oh1 - base, ow0:ow1, :],
                                wt[:, i, :],
                                k1t[:, oh0 + kh - 3:oh1 + kh - 3,
                                    ow0 + kw - 3:ow1 + kw - 3, :],
                                start=(k == 0),
                                stop=(k == len(sub) - 1),
                            )
                    ota = iopool.tile([NPART, FA], f16, tag="ota")
                    nc.vector.tensor_mul(ota[:, :], psa[:, :], sht[:, 0:FA])
                    nc.gpsimd.dma_start(out=odram[it, :, 0:FA],
                                        in_=ota[:, :])
                    otb = iopool.tile([NPART, FB], f16, tag="otb")
                    nc.vector.tensor_mul(otb[:, :], psb[:, :],
                                         sht[:, FA:NFREE])
                    # final store on the by-then idle SP (fast HWDGE path)
                    nc.sync.dma_start(out=odram[it, :, FA:NFREE],
                                      in_=otb[:, :])
    nc.compile()
    return nc


def _get_nc(mode=None):
    if mode is None:
        mode = MODE
    if mode not in _CACHE:
        _CACHE[mode] = _build_nc(mode)
    return _CACHE[mode]


def _host_prep(k1, k2, shell, factor):
    """Build the per-core DMA images (see module docstring for layouts)."""
    k1 = np.asarray(k1, np.float32).reshape(128, 128, KS, KS, KS)
    k2 = np.asarray(k2, np.float32).reshape(128, 128, KS, KS, KS)
    shell = np.asarray(shell, np.float32).reshape(128, 128, KS, KS, KS)
    fac = np.float32(np.asarray(factor).reshape(-1)[0])

    # k1 image: [m, pg, (n2,j,dd), (h,w,p)]
    A = k1.reshape(16, NB, NB, 2, NB, KS, KS, KS)      # m,p,pg,n2,j,d,h,w
    K1 = np.ascontiguousarray(
        A.transpose(0, 2, 3, 4, 5, 6, 7, 1)            # m,pg,n2,j,d,h,w,p
    ).reshape(16, NB, NPART, NFREE).astype(np.float16)

    # banded-Toeplitz k2 image: [m, pg, (n2,j,dd), (kh,kw), (n2',q,od)]
    B = k2.reshape(16, NB, NB, 2, NB, KS, KS, KS)      # m,q,pg,n2,j,td,kh,kw
    dd = np.arange(KS)[:, None]
    od = np.arange(KS)[None, :]
    td = dd - od + 3                                   # (dd, od)
    mask = ((td >= 0) & (td < KS)).astype(np.float32)
    tdc = np.clip(td, 0, KS - 1)
    WB = B[:, :, :, :, :, tdc, :, :]                   # m,q,pg,n2,j,dd,od,kh,kw
    WB = WB * mask[None, None, None, None, None, :, :, None, None]
    WBt = WB.transpose(0, 2, 3, 4, 5, 7, 8, 1, 6)      # m,pg,n2,j,dd,kh,kw,q,od
    # full weight image: part=(n2,j,dd), free=(tap, (n2',q,od)) with the
    # tap axis in EXECUTION order (see _taps) and zero cross-pair blocks
    WBt = WBt.reshape(16, NB, 2, NB, KS, NTAP, NB, KS)  # ..,dd,(khkw),q,od
    order = [kh * KS + kw for kh, kw in _taps()]
    WBt = WBt[:, :, :, :, :, order].astype(np.float16)
    Wfull = np.zeros((16, NB, 2, NB, KS, NTAP, 2, NB, KS), np.float16)
    for i in range(2):
        Wfull[:, :, i, :, :, :, i] = WBt[:, :, i]
    W = Wfull.reshape(16, NB, NPART, NTAP * NPART)

    # shell*factor image: [m, pg, (n2,q,od), (oh,ow,p)] fp16
    C = shell.reshape(16, NB, NB, 2, NB, KS, KS, KS)   # m,p,pg,n2,q,od,oh,ow
    SH = np.ascontiguousarray(
        C.transpose(0, 2, 3, 4, 5, 6, 7, 1) * fac      # m,pg,n2,q,od,oh,ow,p
    ).reshape(16, NB, NPART, NFREE).astype(np.float16)

    return W, K1, SH


def _make_in_maps(W, K1, SH):
    maps = []
    for c in range(N_CORES):
        sl = slice(2 * c, 2 * c + 2)
        maps.append({
            "w": np.ascontiguousarray(W[sl]).reshape(NIT, NPART, -1),
            "k1t": np.ascontiguousarray(K1[sl]).reshape(NIT, NPART, NFREE),
            "sh": np.ascontiguousarray(SH[sl]).reshape(NIT, NPART, NFREE),
        })
    return maps


def _gather(results):
    outs = [np.asarray(r["out"]) for r in results]      # each [16, 112, 392]
    full = np.stack(outs, axis=0).reshape(
        N_CORES, 2, NB, 2, NB, KS, KS, KS, NB
    )                                                   # c,m2,pg,n2,q,od,oh,ow,p
    full = full.transpose(0, 1, 8, 2, 3, 4, 5, 6, 7)    # c,m2,p,pg,n2,q,od,oh,ow
    return np.ascontiguousarray(full).reshape(
        128, 128, KS, KS, KS
    ).astype(np.float32)


def kernel(k1, k2, shell, factor, _trace=False):
    from concourse.bass_utils import run_bass_kernel_spmd

    nc = _get_nc(MODE)
    W, K1, SH = _host_prep(k1, k2, shell, factor)
    in_maps = _make_in_maps(W, K1, SH)
    try:
        res = run_bass_kernel_spmd(
            nc, in_maps, core_ids=list(range(N_CORES)), trace=_trace
        )
    except ModuleNotFoundError:
        res = run_bass_kernel_spmd(
            nc, in_maps, core_ids=list(range(N_CORES)), trace=False
        )
    out = _gather(res.results)
    if _trace:
        return out, res
    return out


# revision 66
# speedup vs baseline: 1.0110x; 1.0083x over previous
"""Trainium2 Bass kernel for nn_ComposedCliffordSteerableKernel.

Computation (see reference): for each of 16x16 (m, n) block pairs, a tiny
3D conv (8,8,7^3) x (8,8,7^3) -> (8,8,7^3) with SAME padding, then
elementwise * shell * factor.

Scheme ("band16"): the TRN2 cost of a matmul is free_size x pe_cycle x
cycles_per_row, independent of how many partitions/columns are used.  So
the kernel packs everything except the (oh, ow, p) free rows into the
partition/column dims:

  psum[(n2,q,od), (oh,ow,p)] += sum_{(n2,j,dd)}
      W[(n2,j,dd), (kh,kw), (n2,q,od)] * k1t[(n2,j,dd), (oh+kh-3, ow+kw-3, p)]

- columns = (pair n2, out-blade q, out-depth od) = 112 of 128
- contraction = (pair n2, in-blade j, ABSOLUTE in-depth dd) = 112
- W is a banded-Toeplitz expansion of k2: W[.., dd, .., od] =
  k2[q, j, dd-od+3, kh, kw] (zero outside the band / across pairs),
  prepared on the host.  The whole depth-tap (kd) sum collapses into the
  dd contraction, so only 49 matmuls per (m, pair-group) remain: one per
  (kh, kw), each streaming (oh,ow,p) rows windowed to the valid
  oh in [max(0,3-kh), min(7,10-kh)) x ow window (rows outside the window
  get zero contribution from this tap).  Tap (3,3) has a full window and
  goes first (start=True initializes the whole accumulator).

Per core: 2 m-blocks x 8 pair-groups = 16 iterations, 49 matmuls each =
784 matmuls, ~175K charged PE rows (~73 us at 2.4 GHz).  All operands
fp16 (1 cycle/row), host-pre-transposed so every DMA is a contiguous
[112, X] block load (the BIR verifier requires the stationary matmul AP
to have a single free dim, so the 112 columns of each tap are stored
contiguously, cross-pair zeros included).  A chained burst of dummy
matmuls while the first weight DMA is in flight ramps the PE to full
clock.  Engine assignment keeps every queue unblocked: weights stream
on SP (pure prefetch, depth 6), k1/shell on Activation, out-DMAs on
Pool/SWDGE (an SP out-DMA would block in-order on the evacuation sem
and collapse the weight prefetch) with the final one on the by-then
idle SP.  shell*factor is folded on the host; one DVE multiply per
iteration evacuates PSUM through 6 rotating banks.  Sharding: core c
takes m-blocks 2c, 2c+1 (output rows 16c..16c+16); no inter-core
communication.  TimelineSim: 84.5 us vs 5469 us baseline (64.7x).
"""

import sys

for _p in ("/opt/trn_rl_repo",):
    if _p not in sys.path:
        sys.path.insert(0, _p)

import numpy as np

NB = 8
KS = 7
S3 = KS * KS * KS          # 343
N_CORES = 8
M_PER_CORE = 2
NIT = 16                   # (m2, pg) iterations per core
NPART = 112                # (n2, j|q, dd|od)
NFREE = KS * KS * NB       # 392 = (oh, ow, p)
NTAP = KS * KS             # 49 (kh, kw)

MODE = "band16"

_CACHE = {}


def _taps():
    """(kh, kw) order: full-window (3,3) first so start=True covers the
    whole accumulator; the rest in raster order."""
    rest = [(kh, kw) for kh in range(KS) for kw in range(KS)
            if (kh, kw) != (3, 3)]
    return [(3, 3)] + rest


def _build_nc(mode):
    import concourse.tile as tile
    from concourse import bacc, mybir

    f16 = mybir.dt.float16
    f32 = mybir.dt.float32

    nc = bacc.Bacc("TRN2", target_bir_lowering=False, debug=False)

    wdram = nc.dram_tensor("w", [NIT, NPART, NTAP * NPART], f16,
                           kind="ExternalInput")
    k1dram = nc.dram_tensor("k1t", [NIT, NPART, NFREE], f16,
                            kind="ExternalInput")
    shdram = nc.dram_tensor("sh", [NIT, NPART, NFREE], f16,
                            kind="ExternalInput")
    odram = nc.dram_tensor("out", [NIT, NPART, NFREE], f16,
                           kind="ExternalOutput")

    taps = _taps()
    WBUFS = 6
    NWARM = 12

    with tile.TileContext(nc) as tc:
        with (
            tc.tile_pool(name="w", bufs=WBUFS) as wpool,
            tc.tile_pool(name="io", bufs=4) as iopool,
            tc.tile_pool(name="shp", bufs=2) as shpool,
            tc.tile_pool(name="ps", bufs=5, space="PSUM") as pspool,
            tc.tile_pool(name="wm", bufs=1) as warmpool,
            tc.tile_pool(name="wps", bufs=1, space="PSUM") as wpspool,
            tc.tile_pool(name="psab", bufs=1, space="PSUM") as psabpool,
        ):
            # PE clock warmup: dummy matmuls on a zero tile while the
            # first weight DMA is in flight (result never read); memset
            # via Pool to keep DVE free
            warm = warmpool.tile([128, 512], f16, tag="warm")
            nc.gpsimd.memset(warm[:, :], 0.0)
            wps = wpspool.tile([128, 512], f32, tag="wps")
            for r in range(NWARM):
                # one long accumulation chain: no intermediate sems, so
                # the PE stays back-to-back busy and actually ramps
                nc.tensor.matmul(wps[:, :], warm[:, 0:128], warm[:, :],
                                 start=(r == 0), stop=(r == NWARM - 1))

            for it in range(NIT):
                # weight tile free layout: (tap, (n2',q,od)) — the 112
                # columns of each tap are contiguous, so the matmul's
                # stationary AP has a single free dim (BIR requirement)
                wt = wpool.tile([NPART, NTAP, NPART], f16, tag="w")
                nc.sync.dma_start(
                    out=wt.rearrange("c a b -> c (a b)"),
                    in_=wdram[it, :, :],
                )
                k1t = iopool.tile([NPART, KS, KS, NB], f16, tag="k1")
                nc.scalar.dma_start(
                    out=k1t.rearrange("c a b p -> c (a b p)"),
                    in_=k1dram[it, :, :],
                )
                sht = shpool.tile([NPART, NFREE], f16, tag="sh")
                nc.scalar.dma_start(out=sht[:, :], in_=shdram[it, :, :])

                if it < NIT - 1:
                    ps = pspool.tile([NPART, NFREE], f32, tag="ps")
                    psv = ps.rearrange("c (oh ow p) -> c oh ow p",
                                       oh=KS, ow=KS)
                    for i, (kh, kw) in enumerate(taps):
                        oh0, oh1 = max(0, 3 - kh), min(KS, 10 - kh)
                        ow0, ow1 = max(0, 3 - kw), min(KS, 10 - kw)
                        nc.tensor.matmul(
                            psv[:, oh0:oh1, ow0:ow1, :],
                            wt[:, i, :],     # tap axis is execution-ordered
                            k1t[:, oh0 + kh - 3:oh1 + kh - 3,
                                ow0 + kw - 3:ow1 + kw - 3, :],
                            start=(i == 0),
                            stop=(i == len(taps) - 1),
                        )
                    ot = iopool.tile([NPART, NFREE], f16, tag="ot")
                    nc.vector.tensor_mul(ot[:, :], ps[:, :], sht[:, :])
                    # out DMAs go via Pool/SWDGE: SP.SEQ stays a pure
                    # weight prefetch stream (an SP out-DMA would block
                    # in-order on the evacuation sem and collapse the
                    # prefetch depth)
                    nc.gpsimd.dma_start(out=odram[it, :, :], in_=ot[:, :])
                else:
                    # last iteration: split into oh-half accumulation
                    # chains A (oh 0:4) and B (oh 4:7), all A matmuls
                    # first — A's evacuation+store overlap B's compute so
                    # the drain tail only pays for the small B half
                    OHA = 5
                    FA, FB = OHA * KS * NB, (KS - OHA) * KS * NB
                    psa = psabpool.tile([NPART, FA], f32, tag="psa")
                    psb = psabpool.tile([NPART, FB], f32, tag="psb")
                    pva = psa.rearrange("c (oh ow p) -> c oh ow p", oh=OHA,
                                        ow=KS)
                    pvb = psb.rearrange("c (oh ow p) -> c oh ow p",
                                        oh=KS - OHA, ow=KS)
                    halves = []
                    for lo, hi, pv, base in ((0, OHA, pva, 0),
                                             (OHA, KS, pvb, OHA)):
                        sub = []
                        for i, (kh, kw) in enumerate(taps):
                            oh0 = max(max(0, 3 - kh), lo)
                            oh1 = min(min(KS, 10 - kh), hi)
                            if oh0 >= oh1:
                                continue
                            ow0, ow1 = max(0, 3 - kw), min(KS, 10 - kw)
                            sub.append((i, kh, kw, oh0, oh1, ow0, ow1))
                        halves.append((pv, base, sub))
                    for pv, base, sub in halves:
                        for k, (i, kh, kw, oh0, oh1, ow0, ow1) in                                 enumerate(sub):
                            nc.tensor.matmul(
                                pv[:, oh0 - base:oh1 - base, ow0:ow1, :],
                                wt[:, i, :],
                                k1t[:, oh0 + kh - 3:oh1 + kh - 3,
                                    ow0 + kw - 3:ow1 + kw - 3, :],
                                start=(k == 0),
                                stop=(k == len(sub) - 1),
                            )
                    ota = iopool.tile([NPART, FA], f16, tag="ota")
                    nc.vector.tensor_mul(ota[:, :], psa[:, :], sht[:, 0:FA])
                    nc.gpsimd.dma_start(out=odram[it, :, 0:FA],
                                        in_=ota[:, :])
                    otb = iopool.tile([NPART, FB], f16, tag="otb")
                    nc.vector.tensor_mul(otb[:, :], psb[:, :],
                                         sht[:, FA:NFREE])
                    # final store on the by-then idle SP (fast HWDGE path)
                    nc.sync.dma_start(out=odram[it, :, FA:NFREE],
                                      in_=otb[:, :])
    nc.compile()
    return nc


def _get_nc(mode=None):
    if mode is None:
        mode = MODE
    if mode not in _CACHE:
        _CACHE[mode] = _build_nc(mode)
    return _CACHE[mode]


def _host_prep(k1, k2, shell, factor):
    """Build the per-core DMA images (see module docstring for layouts)."""
    k1 = np.asarray(k1, np.float32).reshape(128, 128, KS, KS, KS)
    k2 = np.asarray(k2, np.float32).reshape(128, 128, KS, KS, KS)
    shell = np.asarray(shell, np.float32).reshape(128, 128, KS, KS, KS)
    fac = np.float32(np.asarray(factor).reshape(-1)[0])

    # k1 image: [m, pg, (n2,j,dd), (h,w,p)]
    A = k1.reshape(16, NB, NB, 2, NB, KS, KS, KS)      # m,p,pg,n2,j,d,h,w
    K1 = np.ascontiguousarray(
        A.transpose(0, 2, 3, 4, 5, 6, 7, 1)            # m,pg,n2,j,d,h,w,p
    ).reshape(16, NB, NPART, NFREE).astype(np.float16)

    # banded-Toeplitz k2 image: [m, pg, (n2,j,dd), (kh,kw), (n2',q,od)]
    B = k2.reshape(16, NB, NB, 2, NB, KS, KS, KS)      # m,q,pg,n2,j,td,kh,kw
    dd = np.arange(KS)[:, None]
    od = np.arange(KS)[None, :]
    td = dd - od + 3                                   # (dd, od)
    mask = ((td >= 0) & (td < KS)).astype(np.float32)
    tdc = np.clip(td, 0, KS - 1)
    WB = B[:, :, :, :, :, tdc, :, :]                   # m,q,pg,n2,j,dd,od,kh,kw
    WB = WB * mask[None, None, None, None, None, :, :, None, None]
    WBt = WB.transpose(0, 2, 3, 4, 5, 7, 8, 1, 6)      # m,pg,n2,j,dd,kh,kw,q,od
    # full weight image: part=(n2,j,dd), free=(tap, (n2',q,od)) with the
    # tap axis in EXECUTION order (see _taps) and zero cross-pair blocks
    WBt = WBt.reshape(16, NB, 2, NB, KS, NTAP, NB, KS)  # ..,dd,(khkw),q,od
    order = [kh * KS + kw for kh, kw in _taps()]
    WBt = WBt[:, :, :, :, :, order].astype(np.float16)
    Wfull = np.zeros((16, NB, 2, NB, KS, NTAP, 2, NB, KS), np.float16)
    for i in range(2):
        Wfull[:, :, i, :, :, :, i] = WBt[:, :, i]
    W = Wfull.reshape(16, NB, NPART, NTAP * NPART)

    # shell*factor image: [m, pg, (n2,q,od), (oh,ow,p)] fp16
    C = shell.reshape(16, NB, NB, 2, NB, KS, KS, KS)   # m,p,pg,n2,q,od,oh,ow
    SH = np.ascontiguousarray(
        C.transpose(0, 2, 3, 4, 5, 6, 7, 1) * fac      # m,pg,n2,q,od,oh,ow,p
    ).reshape(16, NB, NPART, NFREE).astype(np.float16)

    return W, K1, SH


def _make_in_maps(W, K1, SH):
    maps = []
    for c in range(N_CORES):
        sl = slice(2 * c, 2 * c + 2)
        maps.append({
            "w": np.ascontiguousarray(W[sl]).reshape(NIT, NPART, -1),
            "k1t": np.ascontiguousarray(K1[sl]).reshape(NIT, NPART, NFREE),
            "sh": np.ascontiguousarray(SH[sl]).reshape(NIT, NPART, NFREE),
        })
    return maps


def _gather(results):
    outs = [np.asarray(r["out"]) for r in results]      # each [16, 112, 392]
    full = np.stack(outs, axis=0).reshape(
        N_CORES, 2, NB, 2, NB, KS, KS, KS, NB
    )                                                   # c,m2,pg,n2,q,od,oh,ow,p
    full = full.transpose(0, 1, 8, 2, 3, 4, 5, 6, 7)    # c,m2,p,pg,n2,q,od,oh,ow
    return np.ascontiguousarray(full).reshape(
        128, 128, KS, KS, KS
    ).astype(np.float32)


def kernel(k1, k2, shell, factor, _trace=False):
    from concourse.bass_utils import run_bass_kernel_spmd

    nc = _get_nc(MODE)
    W, K1, SH = _host_prep(k1, k2, shell, factor)
    in_maps = _make_in_maps(W, K1, SH)
    try:
        res = run_bass_kernel_spmd(
            nc, in_maps, core_ids=list(range(N_CORES)), trace=_trace
        )
    except ModuleNotFoundError:
        res = run_bass_kernel_spmd(
            nc, in_maps, core_ids=list(range(N_CORES)), trace=False
        )
    out = _gather(res.results)
    if _trace:
        return out, res
    return out


# revision 72
# speedup vs baseline: 1.0430x; 1.0317x over previous
"""Trainium2 Bass kernel for nn_ComposedCliffordSteerableKernel.

Computation (see reference): for each of 16x16 (m, n) block pairs, a tiny
3D conv (8,8,7^3) x (8,8,7^3) -> (8,8,7^3) with SAME padding, then
elementwise * shell * factor.

Scheme ("band16"): the TRN2 cost of a matmul is free_size x pe_cycle x
cycles_per_row, independent of how many partitions/columns are used.  So
the kernel packs everything except the (oh, ow, p) free rows into the
partition/column dims:

  psum[(n2,q,od), (oh,ow,p)] += sum_{(n2,j,dd)}
      W[(n2,j,dd), (kh,kw), (n2,q,od)] * k1t[(n2,j,dd), (oh+kh-3, ow+kw-3, p)]

- columns = (pair n2, out-blade q, out-depth od) = 112 of 128
- contraction = (pair n2, in-blade j, ABSOLUTE in-depth dd) = 112
- W is a banded-Toeplitz expansion of k2: W[.., dd, .., od] =
  k2[q, j, dd-od+3, kh, kw] (zero outside the band / across pairs),
  prepared on the host.  The whole depth-tap (kd) sum collapses into the
  dd contraction, so only 49 matmuls per (m, pair-group) remain: one per
  (kh, kw), each streaming (oh,ow,p) rows windowed to the valid
  oh in [max(0,3-kh), min(7,10-kh)) x ow window (rows outside the window
  get zero contribution from this tap).  Tap (3,3) has a full window and
  goes first (start=True initializes the whole accumulator).

Per core: 2 m-blocks x 8 pair-groups = 16 iterations, 49 matmuls each =
784 matmuls, ~175K charged PE rows (~73 us at 2.4 GHz).  All operands
fp16 (1 cycle/row), host-pre-transposed so every DMA is a contiguous
[112, X] block load (the BIR verifier requires the stationary matmul AP
to have a single free dim, so the 112 columns of each tap are stored
contiguously, cross-pair zeros included).  A chained burst of dummy
matmuls while the first weight DMA is in flight ramps the PE to full
clock.  Engine assignment keeps every queue unblocked: weights stream
on SP (pure prefetch, depth 6), k1/shell on Activation, out-DMAs on
Pool/SWDGE (an SP out-DMA would block in-order on the evacuation sem
and collapse the weight prefetch) with the final one on the by-then
idle SP.  shell*factor is folded on the host; one DVE multiply per
iteration evacuates PSUM through 6 rotating banks.  Sharding: core c
takes m-blocks 2c, 2c+1 (output rows 16c..16c+16); no inter-core
communication.  TimelineSim: 84.5 us vs 5469 us baseline (64.7x).
"""

import sys

for _p in ("/opt/trn_rl_repo",):
    if _p not in sys.path:
        sys.path.insert(0, _p)

import numpy as np

NB = 8
KS = 7
S3 = KS * KS * KS          # 343
N_CORES = 8
M_PER_CORE = 2
NIT = 16                   # (m2, pg) iterations per core
NPART = 112                # (n2, j|q, dd|od)
NFREE = KS * KS * NB       # 392 = (oh, ow, p)
NTAP = KS * KS             # 49 (kh, kw)

MODE = "band16"

_CACHE = {}


def _taps():
    """(kh, kw) order: full-window (3,3) first so start=True covers the
    whole accumulator; the rest in raster order."""
    rest = [(kh, kw) for kh in range(KS) for kw in range(KS)
            if (kh, kw) != (3, 3)]
    return [(3, 3)] + rest


def _build_nc(mode):
    import concourse.tile as tile
    from concourse import bacc, mybir

    f16 = mybir.dt.float16
    f32 = mybir.dt.float32

    nc = bacc.Bacc("TRN2", target_bir_lowering=False, debug=False)

    wdram = nc.dram_tensor("w", [NIT, NPART, NTAP * NPART], f16,
                           kind="ExternalInput")
    k1dram = nc.dram_tensor("k1t", [NIT, NPART, NFREE], f16,
                            kind="ExternalInput")
    shdram = nc.dram_tensor("sh", [NIT, NPART, NFREE], f16,
                            kind="ExternalInput")
    odram = nc.dram_tensor("out", [NIT, NPART, NFREE], f16,
                           kind="ExternalOutput")

    taps = _taps()
    WBUFS = 6
    NWARM = 7
    PIECES0 = [(0, 16), (16, 28), (28, NTAP)]
    PIECES = [(0, 28), (28, NTAP)]

    with tile.TileContext(nc) as tc:
        with (
            tc.tile_pool(name="w", bufs=WBUFS) as wpool,
            tc.tile_pool(name="io", bufs=4) as iopool,
            tc.tile_pool(name="shp", bufs=2) as shpool,
            tc.tile_pool(name="ps", bufs=5, space="PSUM") as pspool,
            tc.tile_pool(name="wm", bufs=1) as warmpool,
            tc.tile_pool(name="wps", bufs=1, space="PSUM") as wpspool,
            tc.tile_pool(name="psab", bufs=1, space="PSUM") as psabpool,
        ):
            # PE clock warmup: dummy matmuls on a zero tile while the
            # first weight DMA is in flight (result never read); memset
            # via Pool to keep DVE free
            warm = warmpool.tile([128, 512], f16, tag="warm")
            nc.gpsimd.memset(warm[:, :], 0.0)
            wps = wpspool.tile([128, 512], f32, tag="wps")
            for r in range(NWARM):
                # one long accumulation chain: no intermediate sems, so
                # the PE stays back-to-back busy and actually ramps
                nc.tensor.matmul(wps[:, :], warm[:, 0:128], warm[:, :],
                                 start=(r == 0), stop=(r == NWARM - 1))

            for it in range(NIT):
                # weight tile free layout: (tap, (n2',q,od)) — the 112
                # columns of each tap are contiguous, so the matmul's
                # stationary AP has a single free dim (BIR requirement)
                wt = wpool.tile([NPART, NTAP, NPART], f16, tag="w")
                if it < 5:
                    # split early weight loads along the exec-ordered tap
                    # axis so each chain can start once its first piece
                    # lands (the split costs nothing: same bytes, still
                    # >=512B contiguous runs)
                    wv = wdram[it, :, :].rearrange("c (a b) -> c a b",
                                                   a=NTAP)
                    for t0, t1 in (PIECES0 if it == 0 else PIECES):
                        nc.sync.dma_start(out=wt[:, t0:t1, :],
                                          in_=wv[:, t0:t1, :])
                else:
                    nc.sync.dma_start(
                        out=wt.rearrange("c a b -> c (a b)"),
                        in_=wdram[it, :, :],
                    )
                k1t = iopool.tile([NPART, KS, KS, NB], f16, tag="k1")
                nc.scalar.dma_start(
                    out=k1t.rearrange("c a b p -> c (a b p)"),
                    in_=k1dram[it, :, :],
                )
                sht = shpool.tile([NPART, NFREE], f16, tag="sh")
                nc.scalar.dma_start(out=sht[:, :], in_=shdram[it, :, :])

                if it < NIT - 1:
                    ps = pspool.tile([NPART, NFREE], f32, tag="ps")
                    psv = ps.rearrange("c (oh ow p) -> c oh ow p",
                                       oh=KS, ow=KS)
                    for i, (kh, kw) in enumerate(taps):
                        oh0, oh1 = max(0, 3 - kh), min(KS, 10 - kh)
                        ow0, ow1 = max(0, 3 - kw), min(KS, 10 - kw)
                        nc.tensor.matmul(
                            psv[:, oh0:oh1, ow0:ow1, :],
                            wt[:, i, :],     # tap axis is execution-ordered
                            k1t[:, oh0 + kh - 3:oh1 + kh - 3,
                                ow0 + kw - 3:ow1 + kw - 3, :],
                            start=(i == 0),
                            stop=(i == len(taps) - 1),
                        )
                    ot = iopool.tile([NPART, NFREE], f16, tag="ot")
                    nc.vector.tensor_mul(ot[:, :], ps[:, :], sht[:, :])
                    # out DMAs go via Pool/SWDGE: SP.SEQ stays a pure
                    # weight prefetch stream (an SP out-DMA would block
                    # in-order on the evacuation sem and collapse the
                    # prefetch depth)
                    nc.gpsimd.dma_start(out=odram[it, :, :], in_=ot[:, :])
                else:
                    # last iteration: split into oh-half accumulation
                    # chains A (oh 0:4) and B (oh 4:7), all A matmuls
                    # first — A's evacuation+store overlap B's compute so
                    # the drain tail only pays for the small B half
                    OHA = 5
                    FA, FB = OHA * KS * NB, (KS - OHA) * KS * NB
                    psa = psabpool.tile([NPART, FA], f32, tag="psa")
                    psb = psabpool.tile([NPART, FB], f32, tag="psb")
                    pva = psa.rearrange("c (oh ow p) -> c oh ow p", oh=OHA,
                                        ow=KS)
                    pvb = psb.rearrange("c (oh ow p) -> c oh ow p",
                                        oh=KS - OHA, ow=KS)
                    halves = []
                    for lo, hi, pv, base in ((0, OHA, pva, 0),
                                             (OHA, KS, pvb, OHA)):
                        sub = []
                        for i, (kh, kw) in enumerate(taps):
                            oh0 = max(max(0, 3 - kh), lo)
                            oh1 = min(min(KS, 10 - kh), hi)
                            if oh0 >= oh1:
                                continue
                            ow0, ow1 = max(0, 3 - kw), min(KS, 10 - kw)
                            sub.append((i, kh, kw, oh0, oh1, ow0, ow1))
                        halves.append((pv, base, sub))
                    for pv, base, sub in halves:
                        for k, (i, kh, kw, oh0, oh1, ow0, ow1) in                                 enumerate(sub):
                            nc.tensor.matmul(
                                pv[:, oh0 - base:oh1 - base, ow0:ow1, :],
                                wt[:, i, :],
                                k1t[:, oh0 + kh - 3:oh1 + kh - 3,
                                    ow0 + kw - 3:ow1 + kw - 3, :],
                                start=(k == 0),
                                stop=(k == len(sub) - 1),
                            )
                    ota = iopool.tile([NPART, FA], f16, tag="ota")
                    nc.vector.tensor_mul(ota[:, :], psa[:, :], sht[:, 0:FA])
                    nc.gpsimd.dma_start(out=odram[it, :, 0:FA],
                                        in_=ota[:, :])
                    otb = iopool.tile([NPART, FB], f16, tag="otb")
                    nc.vector.tensor_mul(otb[:, :], psb[:, :],
                                         sht[:, FA:NFREE])
                    # final store on the by-then idle SP (fast HWDGE path)
                    nc.sync.dma_start(out=odram[it, :, FA:NFREE],
                                      in_=otb[:, :])
    nc.compile()
    return nc


def _get_nc(mode=None):
    if mode is None:
        mode = MODE
    if mode not in _CACHE:
        _CACHE[mode] = _build_nc(mode)
    return _CACHE[mode]


def _host_prep(k1, k2, shell, factor):
    """Build the per-core DMA images (see module docstring for layouts)."""
    k1 = np.asarray(k1, np.float32).reshape(128, 128, KS, KS, KS)
    k2 = np.asarray(k2, np.float32).reshape(128, 128, KS, KS, KS)
    shell = np.asarray(shell, np.float32).reshape(128, 128, KS, KS, KS)
    fac = np.float32(np.asarray(factor).reshape(-1)[0])

    # k1 image: [m, pg, (n2,j,dd), (h,w,p)]
    A = k1.reshape(16, NB, NB, 2, NB, KS, KS, KS)      # m,p,pg,n2,j,d,h,w
    K1 = np.ascontiguousarray(
        A.transpose(0, 2, 3, 4, 5, 6, 7, 1)            # m,pg,n2,j,d,h,w,p
    ).reshape(16, NB, NPART, NFREE).astype(np.float16)

    # banded-Toeplitz k2 image: [m, pg, (n2,j,dd), (kh,kw), (n2',q,od)]
    B = k2.reshape(16, NB, NB, 2, NB, KS, KS, KS)      # m,q,pg,n2,j,td,kh,kw
    dd = np.arange(KS)[:, None]
    od = np.arange(KS)[None, :]
    td = dd - od + 3                                   # (dd, od)
    mask = ((td >= 0) & (td < KS)).astype(np.float32)
    tdc = np.clip(td, 0, KS - 1)
    WB = B[:, :, :, :, :, tdc, :, :]                   # m,q,pg,n2,j,dd,od,kh,kw
    WB = WB * mask[None, None, None, None, None, :, :, None, None]
    WBt = WB.transpose(0, 2, 3, 4, 5, 7, 8, 1, 6)      # m,pg,n2,j,dd,kh,kw,q,od
    # full weight image: part=(n2,j,dd), free=(tap, (n2',q,od)) with the
    # tap axis in EXECUTION order (see _taps) and zero cross-pair blocks
    WBt = WBt.reshape(16, NB, 2, NB, KS, NTAP, NB, KS)  # ..,dd,(khkw),q,od
    order = [kh * KS + kw for kh, kw in _taps()]
    WBt = WBt[:, :, :, :, :, order].astype(np.float16)
    Wfull = np.zeros((16, NB, 2, NB, KS, NTAP, 2, NB, KS), np.float16)
    for i in range(2):
        Wfull[:, :, i, :, :, :, i] = WBt[:, :, i]
    W = Wfull.reshape(16, NB, NPART, NTAP * NPART)

    # shell*factor image: [m, pg, (n2,q,od), (oh,ow,p)] fp16
    C = shell.reshape(16, NB, NB, 2, NB, KS, KS, KS)   # m,p,pg,n2,q,od,oh,ow
    SH = np.ascontiguousarray(
        C.transpose(0, 2, 3, 4, 5, 6, 7, 1) * fac      # m,pg,n2,q,od,oh,ow,p
    ).reshape(16, NB, NPART, NFREE).astype(np.float16)

    return W, K1, SH


def _make_in_maps(W, K1, SH):
    maps = []
    for c in range(N_CORES):
        sl = slice(2 * c, 2 * c + 2)
        maps.append({
            "w": np.ascontiguousarray(W[sl]).reshape(NIT, NPART, -1),
            "k1t": np.ascontiguousarray(K1[sl]).reshape(NIT, NPART, NFREE),
            "sh": np.ascontiguousarray(SH[sl]).reshape(NIT, NPART, NFREE),
        })
    return maps


def _gather(results):
    outs = [np.asarray(r["out"]) for r in results]      # each [16, 112, 392]
    full = np.stack(outs, axis=0).reshape(
        N_CORES, 2, NB, 2, NB, KS, KS, KS, NB
    )                                                   # c,m2,pg,n2,q,od,oh,ow,p
    full = full.transpose(0, 1, 8, 2, 3, 4, 5, 6, 7)    # c,m2,p,pg,n2,q,od,oh,ow
    return np.ascontiguousarray(full).reshape(
        128, 128, KS, KS, KS
    ).astype(np.float32)


def kernel(k1, k2, shell, factor, _trace=False):
    from concourse.bass_utils import run_bass_kernel_spmd

    nc = _get_nc(MODE)
    W, K1, SH = _host_prep(k1, k2, shell, factor)
    in_maps = _make_in_maps(W, K1, SH)
    try:
        res = run_bass_kernel_spmd(
            nc, in_maps, core_ids=list(range(N_CORES)), trace=_trace
        )
    except ModuleNotFoundError:
        res = run_bass_kernel_spmd(
            nc, in_maps, core_ids=list(range(N_CORES)), trace=False
        )
    out = _gather(res.results)
    if _trace:
        return out, res
    return out
